# revision 11
# baseline (speedup 1.0000x reference)
"""Trainium2 8-core kernel for nn_Attn_user_47863115547245.

reference:
    proj     = id_emb @ attn_W.T + attn_b                  # [seq, hid]
    energies = w1*(user @ proj.T) + w2*(socail @ proj.T)   # [state, seq]
    out      = softmax(energies, axis=-1)

Algebraic restructuring (exact up to float rounding):
  * linearity: energies = (w1*user + w2*socail) @ proj.T
  * reassociation: combined @ (W @ id.T) == (combined @ W) @ id.T, and
    state(2048) < seq(4096) makes (combined @ W) first strictly cheaper.
  * the bias term contributes combined_i . b, constant along the softmax
    axis -> cancels exactly in softmax; dropped.
  * w_big = the larger of |w1|,|w2| is folded into W on the host;
    the ratio w_small/w_big is applied on-device in the combine step.

Sharding: data-parallel over state rows, 256 rows/core x 8 cores.
id_emb (fp16, pre-transposed, s-block-packed) and W (fp16, pre-scaled)
replicated. Softmax is row-local -> zero collectives.

Matmuls run in fp16 with fp32 PSUM accumulation; the softmax
probabilities are written as fp16 and upcast to fp32 on the host
(softmax outputs are in [0,1]; fp16 rounding adds ~2e-4 relative
error on top of the ~2.5e-3 from fp16 matmul inputs).

Perf structure (from NTFF profiles):
  * every host tensor is packed so each DMA is one instruction with
    8-16KB contiguous per-partition runs; DMAs alternate between the
    two HWDGE rings (sync + scalar), which together run at ~HBM rate.
  * mm1 accumulates h-outer into 8 PSUM banks so it streams behind the
    quarter-granular W DMAs, keeping it off the critical path.
  * mm2 iterates seq-chunk-outer / row-tile-inner so the PE work that
    depends on the last id_emb bytes is minimal.
  * softmax is online over NH seq chunks: exp uses the chunk-local max
    immediately; the final per-row rescale exp(Mh-Mtot)/S is folded
    into a dual-scalar normalize, split across DVE and GpSimd.
  * short garbage-matmul bursts bridge PE idle gaps (HAM clock gate).
"""

import numpy as np

STATE, SEQ, HID = 2048, 4096, 1024
NCORES = 8
ROWS = STATE // NCORES        # 256 state rows per core
P = 128                       # partitions
KT = HID // P                 # 8 contraction tiles
MT = ROWS // P                # 2 output row tiles per core
SB = 512                      # seq block (one fp32 PSUM bank)
ST = SEQ // SB                # 8 seq blocks
NH = 4                        # softmax chunks per row tile (online rescale)
SH = ST // NH                 # seq blocks per chunk
WARMUP_MM = 9                 # PE warmup matmuls (HAM un-throttle)

_graph_cache: dict = {}


def _build(ratio: float, swap: bool):
    """Build the per-core Bass graph.

    cT = (sT * ratio) + uT  (or roles swapped when |w2|>|w1|), fp16
    tmpT[k,m] = sum_h W'[h,k] * cT[h,m]      (mm1, fp16, h-outer)
    E[m,s]    = sum_k tmpT[k,m] * idT[k,s]   (mm2, fp16, s-chunk-outer)
    out[m,s]  = softmax_s(E)                 (fp16 out, online chunks)
    """
    import concourse.bacc as bacc
    import concourse.mybir as mybir
    import concourse.bass as bass
    from concourse import tile

    f32, f16 = mybir.dt.float32, mybir.dt.float16
    AX = mybir.AxisListType.X
    ALU = mybir.AluOpType
    ACTF = mybir.ActivationFunctionType

    nc = bacc.Bacc()

    uT = nc.declare_dram_parameter("uT", [P, KT, ROWS], f16, isOutput=False)
    sT = nc.declare_dram_parameter("sT", [P, KT, ROWS], f16, isOutput=False)
    Wp = nc.declare_dram_parameter("Wp", [P, KT, HID], f16, isOutput=False)
    idT = nc.declare_dram_parameter("idT", [ST, P, KT, SB], f16, isOutput=False)
    out = nc.declare_dram_parameter("out", [ROWS, SEQ], f16, isOutput=True)

    with tile.TileContext(nc) as tc:
        with (
            tc.tile_pool(name="inp", bufs=1) as inp,
            tc.tile_pool(name="work", bufs=1) as work,
            tc.tile_pool(name="psum", bufs=8, space=bass.MemorySpace.PSUM) as psp,
        ):
            # ---- PE warmup: garbage matmuls to lift the HAM clock gate ----
            wgarb = work.tile([P, SB], f16, tag="warmgarb")
            nc.gpsimd.memset(wgarb[:], 0.0)
            wps = psp.tile([P, SB], f32, tag="ps")
            for _ in range(WARMUP_MM):
                nc.tensor.matmul(wps[:], wgarb[:, :P], wgarb[:], start=True, stop=True)

            # ---- input DMAs, alternating between the two HWDGE rings ----
            uT_sb = inp.tile([P, KT, ROWS], f16)
            sT_sb = inp.tile([P, KT, ROWS], f16)
            W_sb = inp.tile([P, KT, HID], f16)
            id_sb = inp.tile([P, ST, KT, SB], f16)

            nc.sync.dma_start(uT_sb[:], uT[:])
            nc.scalar.dma_start(sT_sb[:], sT[:])
            # W quarters pace the h-outer mm1; first/last ride the SWDGE
            # path so they don't queue behind uT/sT on the HWDGE rings.
            for j, eng in enumerate((nc.gpsimd, nc.sync, nc.scalar, nc.gpsimd)):
                eng.dma_start(W_sb[:, 2 * j:2 * j + 2, :], Wp[:, 2 * j:2 * j + 2, :])
            for s in range(ST):
                eng = nc.sync if s % 2 == 0 else nc.scalar
                eng.dma_start(id_sb[:, s, :, :], idT[s])

            # ---- combine: cT = (in0 * ratio) + in1, fp16 ----
            cT_sb = work.tile([P, KT, ROWS], f16)
            for k in range(KT):
                in0 = sT_sb[:, k, :] if not swap else uT_sb[:, k, :]
                in1 = uT_sb[:, k, :] if not swap else sT_sb[:, k, :]
                nc.vector.scalar_tensor_tensor(
                    cT_sb[:, k, :], in0, float(ratio), in1,
                    op0=ALU.mult, op1=ALU.add,
                )

            # ---- mm1 (h-outer): 8 banks accumulate tmpT[kb] over h ----
            tmpT_sb = work.tile([P, KT, ROWS], f16)
            ps1 = [psp.tile([P, SB], f32, tag="ps", name=f"ps1_{i}") for i in range(KT)]
            for h in range(KT):
                for kb in range(KT):
                    nc.tensor.matmul(
                        ps1[kb][:, :ROWS],
                        W_sb[:, h, P * kb:P * (kb + 1)],
                        cT_sb[:, h, :],
                        start=(h == 0), stop=(h == KT - 1),
                    )
            for kb in range(KT):
                nc.vector.tensor_copy(tmpT_sb[:, kb, :], ps1[kb][:, :ROWS])

            # ---- mm2 (seq-chunk-outer, m-inner) + online softmax ----
            pun_sb = work.tile([P, MT, SEQ], f16)
            Mh = [work.tile([P, NH], f32, tag=f"Mh{m}", name=f"Mh{m}") for m in range(MT)]
            Sh = [work.tile([P, NH], f32, tag=f"Sh{m}", name=f"Sh{m}") for m in range(MT)]

            for h in range(NH):
                for m in range(MT):
                    banks = []
                    for si in range(SH):
                        s = h * SH + si
                        ps2 = psp.tile([P, SB], f32, tag="ps")
                        for k in range(KT):
                            nc.tensor.matmul(
                                ps2[:],
                                tmpT_sb[:, k, P * m:P * (m + 1)],
                                id_sb[:, s, k, :],
                                start=(k == 0), stop=(k == KT - 1),
                            )
                        banks.append(ps2)

                    pmax = work.tile([P, SH], f32, tag=f"pmax{m}{h}")
                    for si in range(SH):
                        nc.vector.reduce_max(
                            pmax[:, si:si + 1], banks[si][:], axis=AX)
                    nmx = work.tile([P, 1], f32, tag=f"negmax{m}{h}")
                    nc.vector.tensor_reduce(
                        nmx[:], pmax[:], axis=AX, op=ALU.max, negate=True)
                    nc.vector.tensor_scalar_mul(Mh[m][:, h:h + 1], nmx[:], -1.0)

                    psums = work.tile([P, SH], f32, tag=f"psums{m}{h}")
                    for si in range(SH):
                        s = h * SH + si
                        nc.scalar.activation(
                            pun_sb[:, m, SB * s:SB * (s + 1)],
                            banks[si][:],
                            ACTF.Exp,
                            bias=nmx[:],
                            scale=1.0,
                            accum_out=psums[:, si:si + 1],
                        )
                    nc.vector.reduce_sum(Sh[m][:, h:h + 1], psums[:], axis=AX)

            # ---- per-row rescale + normalize + stream out ----
            for m in range(MT):
                negmtot = work.tile([P, 1], f32, tag=f"negmtot{m}")
                nc.vector.tensor_reduce(
                    negmtot[:], Mh[m][:], axis=AX, op=ALU.max, negate=True)
                eh = work.tile([P, NH], f32, tag=f"eh{m}")
                nc.scalar.activation(
                    eh[:], Mh[m][:], ACTF.Exp, bias=negmtot[:], scale=1.0)
                sehs = work.tile([P, NH], f32, tag=f"sehs{m}")
                nc.vector.tensor_mul(sehs[:], Sh[m][:], eh[:])
                stot = work.tile([P, 1], f32, tag=f"stot{m}")
                nc.vector.reduce_sum(stot[:], sehs[:], axis=AX)
                rinv = work.tile([P, 1], f32, tag=f"rinv{m}")
                nc.vector.reciprocal(rinv[:], stot[:])

                for h in range(NH):
                    chunk = slice(SB * SH * h, SB * SH * (h + 1))
                    # m1's normalizes sit on the critical tail: keep them on
                    # the fast DVE; m0's all go to GpSimd so they never delay
                    # m1's pmax/exp/rescale chain on DVE.
                    veng = nc.gpsimd if m == 0 else nc.vector
                    veng.tensor_scalar(
                        pun_sb[:, m, chunk], pun_sb[:, m, chunk],
                        eh[:, h:h + 1], rinv[:],
                        op0=ALU.mult, op1=ALU.mult,
                    )
                    deng = nc.sync if h % 2 == 0 else nc.scalar
                    deng.dma_start(
                        out[P * m:P * (m + 1), chunk], pun_sb[:, m, chunk])

    nc.compile()
    return nc


def _prepare(user_emb, id_emb, socail_uid_emb, attn_W, w1, w2):
    """Host-side sharding + packing. Returns (ratio, swap, in_maps).

    Packed layouts (per-partition contiguous runs -> few big DMA
    descriptors):
      uT/sT: [128, KT, ROWS]   elem [p,k,m] = x[rows0+m, k*128+p]  (fp16)
      Wp:    [128, KT, HID]    elem [p,h,c] = wbig*W[h*128+p, c]   (fp16)
      idT:   [ST, 128, KT, SB] elem [s,p,k,c] = id[s*512+c, k*128+p] (fp16)
    """
    w1 = float(np.asarray(w1))
    w2 = float(np.asarray(w2))
    swap = abs(w2) > abs(w1)
    wbig = w2 if swap else w1
    wsmall = w1 if swap else w2
    ratio = (wsmall / wbig) if wbig != 0.0 else 0.0

    Wp = (np.float32(wbig) * np.asarray(attn_W, np.float32)).astype(np.float16)
    Wp_pack = np.ascontiguousarray(Wp.reshape(KT, P, HID).transpose(1, 0, 2))

    idh = np.asarray(id_emb, np.float32).astype(np.float16)      # [SEQ, HID]
    idT_pack = np.ascontiguousarray(
        idh.reshape(ST, SB, KT, P).transpose(0, 3, 2, 1)         # [s,p,k,c]
    )

    u = np.asarray(user_emb, np.float32).astype(np.float16)
    s_ = np.asarray(socail_uid_emb, np.float32).astype(np.float16)

    in_maps = []
    for i in range(NCORES):
        rows = slice(ROWS * i, ROWS * (i + 1))
        upack = np.ascontiguousarray(
            u[rows].reshape(ROWS, KT, P).transpose(2, 1, 0))
        spack = np.ascontiguousarray(
            s_[rows].reshape(ROWS, KT, P).transpose(2, 1, 0))
        in_maps.append({
            "uT": upack,
            "sT": spack,
            "Wp": Wp_pack,
            "idT": idT_pack,
        })
    return ratio, swap, in_maps


def kernel(user_emb, id_emb, socail_uid_emb, attn_W, attn_b, w1, w2):
    from concourse.bass_utils import run_bass_kernel_spmd

    ratio, swap, in_maps = _prepare(user_emb, id_emb, socail_uid_emb, attn_W, w1, w2)

    key = (round(ratio, 9), swap)
    nc = _graph_cache.get(key)
    if nc is None:
        nc = _build(ratio, swap)
        _graph_cache[key] = nc

    res = run_bass_kernel_spmd(nc, in_maps, core_ids=list(range(NCORES)))
    return np.concatenate(
        [res.results[i]["out"].astype(np.float32) for i in range(NCORES)], axis=0)


# revision 13
# speedup vs baseline: 1.0151x; 1.0151x over previous
"""Trainium2 8-core kernel for nn_Attn_user_47863115547245.

reference:
    proj     = id_emb @ attn_W.T + attn_b                  # [seq, hid]
    energies = w1*(user @ proj.T) + w2*(socail @ proj.T)   # [state, seq]
    out      = softmax(energies, axis=-1)

Algebraic restructuring (exact up to float rounding):
  * linearity: energies = (w1*user + w2*socail) @ proj.T
  * reassociation: combined @ (W @ id.T) == (combined @ W) @ id.T, and
    state(2048) < seq(4096) makes (combined @ W) first strictly cheaper.
  * the bias term contributes combined_i . b, constant along the softmax
    axis -> cancels exactly in softmax; dropped.
  * w_big = the larger of |w1|,|w2| is folded into W on the host;
    the ratio w_small/w_big is applied on-device in the combine step.

Sharding: data-parallel over state rows, 256 rows/core x 8 cores.
id_emb (fp16, pre-transposed, s-block-packed) and W (fp16, pre-scaled)
replicated. Softmax is row-local -> zero collectives.

Matmuls run in fp16 with fp32 PSUM accumulation; the softmax
probabilities are written as fp16 and upcast to fp32 on the host
(softmax outputs are in [0,1]; fp16 rounding adds ~2e-4 relative
error on top of the ~2.5e-3 from fp16 matmul inputs).

Perf structure (from NTFF profiles):
  * every host tensor is packed so each DMA is one instruction with
    8-16KB contiguous per-partition runs; DMAs alternate between the
    two HWDGE rings (sync + scalar), which together run at ~HBM rate.
  * mm1 accumulates h-outer into 8 PSUM banks so it streams behind the
    quarter-granular W DMAs, keeping it off the critical path.
  * mm2 iterates seq-chunk-outer / row-tile-inner so the PE work that
    depends on the last id_emb bytes is minimal.
  * softmax is online over NH seq chunks: exp uses the chunk-local max
    immediately; the final per-row rescale exp(Mh-Mtot)/S is folded
    into a dual-scalar normalize, split across DVE and GpSimd.
  * short garbage-matmul bursts bridge PE idle gaps (HAM clock gate).
"""

import numpy as np

STATE, SEQ, HID = 2048, 4096, 1024
NCORES = 8
ROWS = STATE // NCORES        # 256 state rows per core
P = 128                       # partitions
KT = HID // P                 # 8 contraction tiles
MT = ROWS // P                # 2 output row tiles per core
SB = 512                      # seq block (one fp32 PSUM bank)
ST = SEQ // SB                # 8 seq blocks
NH = 4                        # softmax chunks per row tile (online rescale)
SH = ST // NH                 # seq blocks per chunk
WARMUP_MM = 13                # PE warmup matmuls (HAM un-throttle)

_graph_cache: dict = {}


def _build(ratio: float, swap: bool):
    """Build the per-core Bass graph.

    cT = (sT * ratio) + uT  (or roles swapped when |w2|>|w1|), fp16
    tmpT[k,m] = sum_h W'[h,k] * cT[h,m]      (mm1, fp16, h-outer)
    E[m,s]    = sum_k tmpT[k,m] * idT[k,s]   (mm2, fp16, s-chunk-outer)
    out[m,s]  = softmax_s(E)                 (fp16 out, online chunks)
    """
    import concourse.bacc as bacc
    import concourse.mybir as mybir
    import concourse.bass as bass
    from concourse import tile

    f32, f16 = mybir.dt.float32, mybir.dt.float16
    AX = mybir.AxisListType.X
    ALU = mybir.AluOpType
    ACTF = mybir.ActivationFunctionType

    nc = bacc.Bacc()

    uT = nc.declare_dram_parameter("uT", [P, KT, ROWS], f16, isOutput=False)
    sT = nc.declare_dram_parameter("sT", [P, KT, ROWS], f16, isOutput=False)
    Wp = nc.declare_dram_parameter("Wp", [P, KT, HID], f16, isOutput=False)
    idT = nc.declare_dram_parameter("idT", [ST, P, KT, SB], f16, isOutput=False)
    out = nc.declare_dram_parameter("out", [ROWS, SEQ], f16, isOutput=True)

    with tile.TileContext(nc) as tc:
        with (
            tc.tile_pool(name="inp", bufs=1) as inp,
            tc.tile_pool(name="work", bufs=1) as work,
            tc.tile_pool(name="psum", bufs=8, space=bass.MemorySpace.PSUM) as psp,
        ):
            # ---- PE warmup: garbage matmuls to lift the HAM clock gate ----
            wgarb = work.tile([P, SB], f16, tag="warmgarb")
            nc.gpsimd.memset(wgarb[:], 0.0)
            wps = psp.tile([P, SB], f32, tag="ps")
            for _ in range(WARMUP_MM):
                nc.tensor.matmul(wps[:], wgarb[:, :P], wgarb[:], start=True, stop=True)

            # ---- input DMAs, alternating between the two HWDGE rings ----
            uT_sb = inp.tile([P, KT, ROWS], f16)
            sT_sb = inp.tile([P, KT, ROWS], f16)
            W_sb = inp.tile([P, KT, HID], f16)
            id_sb = inp.tile([P, ST, KT, SB], f16)

            nc.sync.dma_start(uT_sb[:], uT[:])
            nc.scalar.dma_start(sT_sb[:], sT[:])
            for j in range(4):  # W quarters pace the h-outer mm1
                eng = nc.sync if j % 2 == 0 else nc.scalar
                eng.dma_start(W_sb[:, 2 * j:2 * j + 2, :], Wp[:, 2 * j:2 * j + 2, :])
            for s in range(ST):
                eng = nc.sync if s % 2 == 0 else nc.scalar
                eng.dma_start(id_sb[:, s, :, :], idT[s])

            # ---- combine: cT = (in0 * ratio) + in1, fp16 ----
            cT_sb = work.tile([P, KT, ROWS], f16)
            for k in range(KT):
                in0 = sT_sb[:, k, :] if not swap else uT_sb[:, k, :]
                in1 = uT_sb[:, k, :] if not swap else sT_sb[:, k, :]
                nc.vector.scalar_tensor_tensor(
                    cT_sb[:, k, :], in0, float(ratio), in1,
                    op0=ALU.mult, op1=ALU.add,
                )

            # ---- mm1 (h-outer): 8 banks accumulate tmpT[kb] over h ----
            tmpT_sb = work.tile([P, KT, ROWS], f16)
            ps1 = [psp.tile([P, SB], f32, tag="ps", name=f"ps1_{i}") for i in range(KT)]
            for h in range(KT):
                for kb in range(KT):
                    nc.tensor.matmul(
                        ps1[kb][:, :ROWS],
                        W_sb[:, h, P * kb:P * (kb + 1)],
                        cT_sb[:, h, :],
                        start=(h == 0), stop=(h == KT - 1),
                    )
            for kb in range(KT):
                nc.vector.tensor_copy(tmpT_sb[:, kb, :], ps1[kb][:, :ROWS])

            # ---- mm2 (seq-chunk-outer, m-inner) + online softmax ----
            pun_sb = work.tile([P, MT, SEQ], f16)
            Mh = [work.tile([P, NH], f32, tag=f"Mh{m}", name=f"Mh{m}") for m in range(MT)]
            Sh = [work.tile([P, NH], f32, tag=f"Sh{m}", name=f"Sh{m}") for m in range(MT)]

            def rescale_and_out(m):
                """Per-row rescale + normalize + stream out for row tile m.

                Emitted immediately after tile m's last exp so its ops land
                early in each engine's FIFO (ACT executes strictly in order).
                """
                negmtot = work.tile([P, 1], f32, tag=f"negmtot{m}",
                                    name=f"negmtot{m}")
                nc.vector.tensor_reduce(
                    negmtot[:], Mh[m][:], axis=AX, op=ALU.max, negate=True)
                eh = work.tile([P, NH], f32, tag=f"eh{m}", name=f"eh{m}")
                nc.scalar.activation(
                    eh[:], Mh[m][:], ACTF.Exp, bias=negmtot[:], scale=1.0)
                sehs = work.tile([P, NH], f32, tag=f"sehs{m}", name=f"sehs{m}")
                nc.vector.tensor_mul(sehs[:], Sh[m][:], eh[:])
                stot = work.tile([P, 1], f32, tag=f"stot{m}", name=f"stot{m}")
                nc.vector.reduce_sum(stot[:], sehs[:], axis=AX)
                rinv = work.tile([P, 1], f32, tag=f"rinv{m}", name=f"rinv{m}")
                nc.vector.reciprocal(rinv[:], stot[:])

                for h in range(NH):
                    chunk = slice(SB * SH * h, SB * SH * (h + 1))
                    # m1's normalizes sit on the critical tail: keep them on
                    # the fast DVE; split m0's between DVE and GpSimd.
                    veng = nc.gpsimd if (m == 0 and h % 2 == 1) else nc.vector
                    veng.tensor_scalar(
                        pun_sb[:, m, chunk], pun_sb[:, m, chunk],
                        eh[:, h:h + 1], rinv[:],
                        op0=ALU.mult, op1=ALU.mult,
                    )
                    deng = nc.sync if h % 2 == 0 else nc.scalar
                    deng.dma_start(
                        out[P * m:P * (m + 1), chunk], pun_sb[:, m, chunk])

            for h in range(NH):
                for m in range(MT):
                    banks = []
                    for si in range(SH):
                        s = h * SH + si
                        ps2 = psp.tile([P, SB], f32, tag="ps")
                        for k in range(KT):
                            nc.tensor.matmul(
                                ps2[:],
                                tmpT_sb[:, k, P * m:P * (m + 1)],
                                id_sb[:, s, k, :],
                                start=(k == 0), stop=(k == KT - 1),
                            )
                        banks.append(ps2)

                    pmax = work.tile([P, SH], f32, tag=f"pmax{m}{h}")
                    for si in range(SH):
                        nc.vector.reduce_max(
                            pmax[:, si:si + 1], banks[si][:], axis=AX)
                    nmx = work.tile([P, 1], f32, tag=f"negmax{m}{h}")
                    nc.vector.tensor_reduce(
                        nmx[:], pmax[:], axis=AX, op=ALU.max, negate=True)
                    nc.vector.tensor_scalar_mul(Mh[m][:, h:h + 1], nmx[:], -1.0)

                    psums = work.tile([P, SH], f32, tag=f"psums{m}{h}")
                    for si in range(SH):
                        s = h * SH + si
                        nc.scalar.activation(
                            pun_sb[:, m, SB * s:SB * (s + 1)],
                            banks[si][:],
                            ACTF.Exp,
                            bias=nmx[:],
                            scale=1.0,
                            accum_out=psums[:, si:si + 1],
                        )
                    nc.vector.reduce_sum(Sh[m][:, h:h + 1], psums[:], axis=AX)

                    if h == NH - 1:
                        rescale_and_out(m)

    nc.compile()
    return nc


def _prepare(user_emb, id_emb, socail_uid_emb, attn_W, w1, w2):
    """Host-side sharding + packing. Returns (ratio, swap, in_maps).

    Packed layouts (per-partition contiguous runs -> few big DMA
    descriptors):
      uT/sT: [128, KT, ROWS]   elem [p,k,m] = x[rows0+m, k*128+p]  (fp16)
      Wp:    [128, KT, HID]    elem [p,h,c] = wbig*W[h*128+p, c]   (fp16)
      idT:   [ST, 128, KT, SB] elem [s,p,k,c] = id[s*512+c, k*128+p] (fp16)
    """
    w1 = float(np.asarray(w1))
    w2 = float(np.asarray(w2))
    swap = abs(w2) > abs(w1)
    wbig = w2 if swap else w1
    wsmall = w1 if swap else w2
    ratio = (wsmall / wbig) if wbig != 0.0 else 0.0

    Wp = (np.float32(wbig) * np.asarray(attn_W, np.float32)).astype(np.float16)
    Wp_pack = np.ascontiguousarray(Wp.reshape(KT, P, HID).transpose(1, 0, 2))

    idh = np.asarray(id_emb, np.float32).astype(np.float16)      # [SEQ, HID]
    idT_pack = np.ascontiguousarray(
        idh.reshape(ST, SB, KT, P).transpose(0, 3, 2, 1)         # [s,p,k,c]
    )

    u = np.asarray(user_emb, np.float32).astype(np.float16)
    s_ = np.asarray(socail_uid_emb, np.float32).astype(np.float16)

    in_maps = []
    for i in range(NCORES):
        rows = slice(ROWS * i, ROWS * (i + 1))
        upack = np.ascontiguousarray(
            u[rows].reshape(ROWS, KT, P).transpose(2, 1, 0))
        spack = np.ascontiguousarray(
            s_[rows].reshape(ROWS, KT, P).transpose(2, 1, 0))
        in_maps.append({
            "uT": upack,
            "sT": spack,
            "Wp": Wp_pack,
            "idT": idT_pack,
        })
    return ratio, swap, in_maps


def kernel(user_emb, id_emb, socail_uid_emb, attn_W, attn_b, w1, w2):
    from concourse.bass_utils import run_bass_kernel_spmd

    ratio, swap, in_maps = _prepare(user_emb, id_emb, socail_uid_emb, attn_W, w1, w2)

    key = (round(ratio, 9), swap)
    nc = _graph_cache.get(key)
    if nc is None:
        nc = _build(ratio, swap)
        _graph_cache[key] = nc

    res = run_bass_kernel_spmd(nc, in_maps, core_ids=list(range(NCORES)))
    return np.concatenate(
        [res.results[i]["out"].astype(np.float32) for i in range(NCORES)], axis=0)
